# revision 9
# baseline (speedup 1.0000x reference)
# Trainium2 Bass kernel for nn_LogitsNew (dense_mlp).
#
#   u = gelu(x @ W_proj + b_proj)                       [B, D]
#   logits = (u @ W_u)[:, None, :] + ee @ W_e           [B, N, C]
#
# Sharding: data-parallel over batch B across 8 cores (4 batches/core).
# Per core:
#   - main path: for each 128-row tile of ee, PE-transpose the 8 [128,128]
#     d-chunks, accumulate eeT.T @ W_e into two PSUM banks (c halves) with a
#     shared stationary per k, drain PSUM->SBUF immediately (no y dep).
#   - utterance path: z = x@W_proj (+b via K=1 ones matmul), u = Gelu(z),
#     y = u@W_u, broadcast y across partitions with gpsimd.
#   - epilogue: out_sb += y_bcast, DMA out.
# Matmuls run as float32r (full-rate fp32 path on the PE for free dim >= 256).

import sys

if "/opt/trn_rl_repo" not in sys.path:
    sys.path.insert(0, "/opt/trn_rl_repo")

import numpy as np

import concourse.bass as bass
import concourse.mybir as mybir
import concourse.tile as tile
from concourse import bacc
from concourse.bass_utils import run_bass_kernel_spmd
from concourse.masks import make_identity

P = 128
B, N, D, C = 32, 256, 1024, 1024
NCORES = 8
BPC = B // NCORES          # batches per core
KT = D // P                # 8 k-tiles over the contraction dim
FD = 512                   # matmul moving free dim (one PSUM bank of fp32)
NT = N // P                # 2 n-tiles per batch
MT = BPC * NT              # 8 m-tiles per core

F32 = mybir.dt.float32
F32R = mybir.dt.float32r
GELU = mybir.ActivationFunctionType.Gelu

_CACHE = {}


def _build():
    if "nc" in _CACHE:
        return _CACHE["nc"]

    nc = bacc.Bacc("TRN2", target_bir_lowering=False, debug=False, num_devices=NCORES)

    x = nc.dram_tensor("encoded_utterance", [BPC, D], F32R, kind="ExternalInput").ap()
    ee = nc.dram_tensor(
        "element_embeddings", [BPC, N, D], F32R, kind="ExternalInput"
    ).ap()
    w = nc.dram_tensor("weight_matrix", [2 * D, C], F32R, kind="ExternalInput").ap()
    wp = nc.dram_tensor("W_proj", [D, D], F32R, kind="ExternalInput").ap()
    bp = nc.dram_tensor("b_proj", [1, D], F32R, kind="ExternalInput").ap()
    out = nc.dram_tensor("logits", [BPC, N, C], F32, kind="ExternalOutput").ap()

    w3 = w.rearrange("(ko p) c -> p ko c", p=P)     # [128, 16, 1024]; ko 0..7 = W_u
    wp3 = wp.rearrange("(ko p) c -> p ko c", p=P)   # [128, 8, 1024]

    with tile.TileContext(nc) as tc:
        with (
            tc.tile_pool(name="const", bufs=1) as cpool,
            tc.tile_pool(name="weights", bufs=1) as wpool,
            tc.tile_pool(name="ee", bufs=3) as eepool,
            tc.tile_pool(name="eet", bufs=2) as eetpool,
            tc.tile_pool(name="outs", bufs=1) as outpool,
            tc.tile_pool(name="tp_ps", bufs=2, space="PSUM") as tp_ps,
            tc.tile_pool(name="mm_ps", bufs=6, space="PSUM") as mm_ps,
        ):
            # ---- constants / small inputs ----
            ident_f = cpool.tile([P, P], F32)
            make_identity(nc, ident_f)
            ident = cpool.tile([P, P], F32R)
            nc.scalar.copy(ident, ident_f)
            ones_f = cpool.tile([1, P], F32)
            nc.gpsimd.memset(ones_f, 1.0)
            ones = cpool.tile([1, P], F32R)
            nc.scalar.copy(ones, ones_f)
            x_sb = cpool.tile([BPC, D], F32R)
            nc.sync.dma_start(x_sb, x)
            b_sb = cpool.tile([1, D], F32R)
            nc.sync.dma_start(b_sb, bp)

            # ---- weights: W_e on the SP ring (feeds main loop early);
            # W_proj / W_u on the ACT ring. ----
            w_sb = wpool.tile([P, 2 * KT, C], F32R)
            wp_sb = wpool.tile([P, KT, C], F32R)
            nc.sync.dma_start(w_sb[:, 8:12], w3[:, 8:12])
            nc.sync.dma_start(w_sb[:, 12:16], w3[:, 12:16])
            nc.scalar.dma_start(wp_sb[:, 0:4], wp3[:, 0:4])
            nc.scalar.dma_start(wp_sb[:, 4:8], wp3[:, 4:8])
            nc.scalar.dma_start(w_sb[:, 0:4], w3[:, 0:4])
            nc.scalar.dma_start(w_sb[:, 4:8], w3[:, 4:8])

            # ---- main path: transposes + matmuls + early drains ----
            out_tiles = []
            for mt in range(MT):
                b, nh = divmod(mt, NT)
                ns = slice(nh * P, (nh + 1) * P)
                ee_t = eepool.tile([P, D], F32R, tag="ee")
                nc.sync.dma_start(ee_t, ee[b, ns, :])
                eet = eetpool.tile([P, KT, P], F32R, tag="eet")
                for k in range(KT):
                    tp = tp_ps.tile([P, P], F32R, tag="tp")
                    nc.tensor.transpose(tp, ee_t[:, k * P : (k + 1) * P], ident)
                    if k % 2 == 0:
                        nc.scalar.copy(eet[:, k, :], tp)
                    else:
                        nc.vector.tensor_copy(eet[:, k, :], tp)
                mps = [
                    mm_ps.tile([P, FD], F32, tag="mm", name=f"mm_{mt}_{ch}")
                    for ch in range(2)
                ]
                for k in range(KT):
                    for ch in range(2):
                        nc.tensor.matmul(
                            mps[ch],
                            eet[:, k, :],
                            w_sb[:, KT + k, ch * FD : (ch + 1) * FD],
                            start=(k == 0),
                            stop=(k == KT - 1),
                        )
                o = outpool.tile([P, 2, FD], F32, tag=f"o{mt}")
                nc.scalar.copy(o[:, 0, :], mps[0])
                nc.scalar.copy(o[:, 1, :], mps[1])
                out_tiles.append(o)

            # ---- utterance path ----
            xT = cpool.tile([P, KT, BPC], F32R)
            for k in range(KT):
                tp = tp_ps.tile([P, P], F32R, tag="tp")
                nc.tensor.transpose(
                    tp[:, :BPC], x_sb[:BPC, k * P : (k + 1) * P], ident[:BPC, :BPC]
                )
                nc.scalar.copy(xT[:, k, :], tp[:, :BPC])

            u_sb = cpool.tile([BPC, C], F32R)
            for h in range(2):
                cs = slice(h * FD, (h + 1) * FD)
                zp = mm_ps.tile([P, FD], F32, tag="mm")
                for k in range(KT):
                    nc.tensor.matmul(
                        zp[:BPC], xT[:, k, :], wp_sb[:, k, cs],
                        start=(k == 0), stop=False,
                    )
                nc.tensor.matmul(
                    zp[:BPC], ones[:1, :BPC], b_sb[:1, cs],
                    start=False, stop=True,
                )
                nc.scalar.activation(u_sb[:, cs], zp[:BPC], GELU)

            uT = cpool.tile([P, KT, BPC], F32R)
            for k in range(KT):
                tp = tp_ps.tile([P, P], F32R, tag="tp")
                nc.tensor.transpose(
                    tp[:, :BPC], u_sb[:BPC, k * P : (k + 1) * P], ident[:BPC, :BPC]
                )
                nc.scalar.copy(uT[:, k, :], tp[:, :BPC])

            y_sb = cpool.tile([BPC, C], F32)
            for h in range(2):
                cs = slice(h * FD, (h + 1) * FD)
                yp = mm_ps.tile([P, FD], F32, tag="mm")
                for k in range(KT):
                    nc.tensor.matmul(
                        yp[:BPC], uT[:, k, :], w_sb[:, k, cs],
                        start=(k == 0), stop=(k == KT - 1),
                    )
                nc.vector.tensor_copy(y_sb[:, cs], yp[:BPC])

            # y as a single row so each batch's slice starts at partition 0,
            # then broadcast each batch's y row across all 128 partitions
            y_row = cpool.tile([1, BPC, C], F32)
            nc.scalar.dma_start(y_row, y_sb)
            ybc = cpool.tile([P, BPC, C], F32)
            for b in range(BPC):
                nc.gpsimd.partition_broadcast(ybc[:, b, :], y_row[:1, b, :])

            # ---- epilogue: add broadcast y, store ----
            for mt in range(MT):
                b, nh = divmod(mt, NT)
                ns = slice(nh * P, (nh + 1) * P)
                o = out_tiles[mt]
                nc.vector.tensor_add(o[:, 0, :], o[:, 0, :], ybc[:, b, 0:FD])
                nc.vector.tensor_add(o[:, 1, :], o[:, 1, :], ybc[:, b, FD:C])
                nc.sync.dma_start(out[b, ns, :], o.rearrange("p a f -> p (a f)"))

    nc.compile()
    _CACHE["nc"] = nc
    return nc


def run(inputs, trace=False, **kwargs):
    nc = _build()
    x = np.ascontiguousarray(np.asarray(inputs["encoded_utterance"], np.float32))
    ee = np.ascontiguousarray(np.asarray(inputs["element_embeddings"], np.float32))
    w = np.ascontiguousarray(np.asarray(inputs["weight_matrix"], np.float32))
    wp = np.ascontiguousarray(np.asarray(inputs["W_proj"], np.float32))
    bp = np.ascontiguousarray(
        np.asarray(inputs["b_proj"], np.float32).reshape(1, D)
    )

    in_maps = []
    for i in range(NCORES):
        bs = slice(i * BPC, (i + 1) * BPC)
        in_maps.append(
            {
                "encoded_utterance": x[bs],
                "element_embeddings": ee[bs],
                "weight_matrix": w,
                "W_proj": wp,
                "b_proj": bp,
            }
        )

    res = run_bass_kernel_spmd(
        nc, in_maps, core_ids=list(range(NCORES)), trace=trace, **kwargs
    )
    full = np.concatenate([r["logits"] for r in res.results], axis=0)
    return full, res


def kernel(**inputs) -> np.ndarray:
    return run(inputs, trace=False)[0]


# revision 10
# speedup vs baseline: 1.2933x; 1.2933x over previous
# Trainium2 Bass kernel for nn_LogitsNew (dense_mlp).
#
#   u = gelu(x @ W_proj + b_proj)                       [B, D]
#   logits = (u @ W_u)[:, None, :] + ee @ W_e           [B, N, C]
#
# Sharding: data-parallel over batch B across 8 cores (4 batches/core).
# Per core:
#   - main path: for each 128-row tile of ee, PE-transpose the 8 [128,128]
#     d-chunks, accumulate eeT.T @ W_e into two PSUM banks (c halves),
#     drain PSUM->SBUF immediately (no dependency on the utterance path).
#   - utterance path (emitted mid-loop, after its weights have landed):
#     z = x@W_proj (+b via K=1 ones matmul), u = Gelu(z), y = u@W_u,
#     broadcast y across partitions with gpsimd.
#   - epilogue: out_sb += y_bcast on DVE, DMA out.
# Matmuls run as float32r (full-rate fp32 path on the PE for free dim >= 256).
#
# DMA ring usage: SP ring carries ee[0..3] + all weights (in the order the
# PE consumes them: W_e, W_proj, W_u) + output stores; ACT ring carries
# ee[4..7] + small transfers. Engines execute their streams in order, so
# program order tracks data-arrival order.

import sys

if "/opt/trn_rl_repo" not in sys.path:
    sys.path.insert(0, "/opt/trn_rl_repo")

import numpy as np

import concourse.bass as bass
import concourse.mybir as mybir
import concourse.tile as tile
from concourse import bacc
from concourse.bass_utils import run_bass_kernel_spmd
from concourse.masks import make_identity

P = 128
B, N, D, C = 32, 256, 1024, 1024
NCORES = 8
BPC = B // NCORES          # batches per core
KT = D // P                # 8 k-tiles over the contraction dim
FD = 512                   # matmul moving free dim (one PSUM bank of fp32)
NT = N // P                # 2 n-tiles per batch
MT = BPC * NT              # 8 m-tiles per core

F32 = mybir.dt.float32
F32R = mybir.dt.float32r
GELU = mybir.ActivationFunctionType.Gelu

_CACHE = {}


def _build():
    if "nc" in _CACHE:
        return _CACHE["nc"]

    nc = bacc.Bacc("TRN2", target_bir_lowering=False, debug=False, num_devices=NCORES)

    x = nc.dram_tensor("encoded_utterance", [BPC, D], F32R, kind="ExternalInput").ap()
    ee = nc.dram_tensor(
        "element_embeddings", [BPC, N, D], F32R, kind="ExternalInput"
    ).ap()
    w = nc.dram_tensor("weight_matrix", [2 * D, C], F32R, kind="ExternalInput").ap()
    wp = nc.dram_tensor("W_proj", [D, D], F32R, kind="ExternalInput").ap()
    bp = nc.dram_tensor("b_proj", [1, D], F32R, kind="ExternalInput").ap()
    out = nc.dram_tensor("logits", [BPC, N, C], F32, kind="ExternalOutput").ap()

    w3 = w.rearrange("(ko p) c -> p ko c", p=P)     # [128, 16, 1024]; ko 0..7 = W_u
    wp3 = wp.rearrange("(ko p) c -> p ko c", p=P)   # [128, 8, 1024]

    with tile.TileContext(nc) as tc:
        with (
            tc.tile_pool(name="const", bufs=1) as cpool,
            tc.tile_pool(name="weights", bufs=1) as wpool,
            tc.tile_pool(name="ee", bufs=4) as eepool,
            tc.tile_pool(name="eet", bufs=2) as eetpool,
            tc.tile_pool(name="outs", bufs=1) as outpool,
            tc.tile_pool(name="tp_ps", bufs=2, space="PSUM") as tp_ps,
            tc.tile_pool(name="mm_ps", bufs=6, space="PSUM") as mm_ps,
        ):
            # ---- constants / small inputs ----
            ident_f = cpool.tile([P, P], F32)
            make_identity(nc, ident_f)
            ident = cpool.tile([P, P], F32R)
            nc.scalar.copy(ident, ident_f)
            ones_f = cpool.tile([1, P], F32)
            nc.gpsimd.memset(ones_f, 1.0)
            ones = cpool.tile([1, P], F32R)
            nc.scalar.copy(ones, ones_f)
            x_sb = cpool.tile([BPC, D], F32R)
            nc.sync.dma_start(x_sb, x)
            b_sb = cpool.tile([1, D], F32R)
            nc.sync.dma_start(b_sb, bp)

            # ---- first 4 ee tiles on the SP ring, ahead of the weights ----
            ee_tiles = {}
            for mt in range(4):
                b, nh = divmod(mt, NT)
                ee_t = eepool.tile([P, D], F32R, tag="ee", name=f"ee_{mt}")
                nc.sync.dma_start(ee_t, ee[b, nh * P : (nh + 1) * P, :])
                ee_tiles[mt] = ee_t

            # ---- weights on the SP ring, 1MB slices, in consumption order ----
            w_sb = wpool.tile([P, 2 * KT, C], F32R)
            wp_sb = wpool.tile([P, KT, C], F32R)
            for j in range(4):  # W_e
                nc.sync.dma_start(
                    w_sb[:, 8 + 2 * j : 10 + 2 * j], w3[:, 8 + 2 * j : 10 + 2 * j]
                )
            for j in range(4):  # W_proj
                nc.sync.dma_start(wp_sb[:, 2 * j : 2 * j + 2], wp3[:, 2 * j : 2 * j + 2])
            for j in range(4):  # W_u
                nc.sync.dma_start(w_sb[:, 2 * j : 2 * j + 2], w3[:, 2 * j : 2 * j + 2])

            # ---- main path (utterance path spliced in after m-tile 3) ----
            out_tiles = []
            u_sb = None
            for mt in range(MT):
                if mt == 4:
                    # ---- utterance path: by now W_proj/W_u have landed ----
                    xT = cpool.tile([P, KT, BPC], F32R)
                    for k in range(KT):
                        tp = tp_ps.tile([P, P], F32R, tag="tp")
                        nc.tensor.transpose(
                            tp[:, :BPC],
                            x_sb[:BPC, k * P : (k + 1) * P],
                            ident[:BPC, :BPC],
                        )
                        nc.scalar.copy(xT[:, k, :], tp[:, :BPC])

                    u_sb = cpool.tile([BPC, C], F32R)
                    for h in range(2):
                        cs = slice(h * FD, (h + 1) * FD)
                        zp = mm_ps.tile([P, FD], F32, tag="mm", name=f"z_{h}")
                        for k in range(KT):
                            nc.tensor.matmul(
                                zp[:BPC], xT[:, k, :], wp_sb[:, k, cs],
                                start=(k == 0), stop=False,
                            )
                        nc.tensor.matmul(
                            zp[:BPC], ones[:1, :BPC], b_sb[:1, cs],
                            start=False, stop=True,
                        )
                        nc.scalar.activation(u_sb[:, cs], zp[:BPC], GELU)

                    uT = cpool.tile([P, KT, BPC], F32R)
                    for k in range(KT):
                        tp = tp_ps.tile([P, P], F32R, tag="tp")
                        nc.tensor.transpose(
                            tp[:, :BPC],
                            u_sb[:BPC, k * P : (k + 1) * P],
                            ident[:BPC, :BPC],
                        )
                        nc.scalar.copy(uT[:, k, :], tp[:, :BPC])

                    y_sb = cpool.tile([BPC, C], F32)
                    for h in range(2):
                        cs = slice(h * FD, (h + 1) * FD)
                        yp = mm_ps.tile([P, FD], F32, tag="mm", name=f"y_{h}")
                        for k in range(KT):
                            nc.tensor.matmul(
                                yp[:BPC], uT[:, k, :], w_sb[:, k, cs],
                                start=(k == 0), stop=(k == KT - 1),
                            )
                        nc.vector.tensor_copy(y_sb[:, cs], yp[:BPC])

                    y_row = cpool.tile([1, BPC, C], F32)
                    nc.scalar.dma_start(y_row, y_sb)
                    ybc = cpool.tile([P, BPC, C], F32)
                    for b2 in range(BPC):
                        nc.gpsimd.partition_broadcast(ybc[:, b2, :], y_row[:1, b2, :])

                b, nh = divmod(mt, NT)
                ns = slice(nh * P, (nh + 1) * P)
                if mt >= 4:
                    ee_t = eepool.tile([P, D], F32R, tag="ee", name=f"ee_{mt}")
                    nc.scalar.dma_start(ee_t, ee[b, ns, :])
                    ee_tiles[mt] = ee_t
                ee_t = ee_tiles[mt]
                eet = eetpool.tile([P, KT, P], F32R, tag="eet")
                for k in range(KT):
                    tp = tp_ps.tile([P, P], F32R, tag="tp")
                    nc.tensor.transpose(tp, ee_t[:, k * P : (k + 1) * P], ident)
                    if k % 2 == 0:
                        nc.scalar.copy(eet[:, k, :], tp)
                    else:
                        nc.vector.tensor_copy(eet[:, k, :], tp)
                mps = [
                    mm_ps.tile([P, FD], F32, tag="mm", name=f"mm_{mt}_{ch}")
                    for ch in range(2)
                ]
                for k in range(KT):
                    for ch in range(2):
                        nc.tensor.matmul(
                            mps[ch],
                            eet[:, k, :],
                            w_sb[:, KT + k, ch * FD : (ch + 1) * FD],
                            start=(k == 0),
                            stop=(k == KT - 1),
                        )
                o = outpool.tile([P, 2, FD], F32, tag=f"o{mt}")
                nc.scalar.copy(o[:, 0, :], mps[0])
                nc.scalar.copy(o[:, 1, :], mps[1])
                out_tiles.append(o)

            # ---- epilogue: add broadcast y, store ----
            for mt in range(MT):
                b, nh = divmod(mt, NT)
                ns = slice(nh * P, (nh + 1) * P)
                o = out_tiles[mt]
                nc.vector.tensor_add(o[:, 0, :], o[:, 0, :], ybc[:, b, 0:FD])
                nc.vector.tensor_add(o[:, 1, :], o[:, 1, :], ybc[:, b, FD:C])
                nc.sync.dma_start(out[b, ns, :], o.rearrange("p a f -> p (a f)"))

    nc.compile()
    _CACHE["nc"] = nc
    return nc


def run(inputs, trace=False, **kwargs):
    nc = _build()
    x = np.ascontiguousarray(np.asarray(inputs["encoded_utterance"], np.float32))
    ee = np.ascontiguousarray(np.asarray(inputs["element_embeddings"], np.float32))
    w = np.ascontiguousarray(np.asarray(inputs["weight_matrix"], np.float32))
    wp = np.ascontiguousarray(np.asarray(inputs["W_proj"], np.float32))
    bp = np.ascontiguousarray(
        np.asarray(inputs["b_proj"], np.float32).reshape(1, D)
    )

    in_maps = []
    for i in range(NCORES):
        bs = slice(i * BPC, (i + 1) * BPC)
        in_maps.append(
            {
                "encoded_utterance": x[bs],
                "element_embeddings": ee[bs],
                "weight_matrix": w,
                "W_proj": wp,
                "b_proj": bp,
            }
        )

    res = run_bass_kernel_spmd(
        nc, in_maps, core_ids=list(range(NCORES)), trace=trace, **kwargs
    )
    full = np.concatenate([r["logits"] for r in res.results], axis=0)
    return full, res


def kernel(**inputs) -> np.ndarray:
    return run(inputs, trace=False)[0]
